# revision 10
# baseline (speedup 1.0000x reference)
"""Trainium2 Bass kernel for nn_BlockConvolutionLean.

Computation (see reference):
  features = einsum('nse,te->nst', seq_vector, W)        # 1x1 conv
  blocks of BS=8 along S; out = exclusive-cumsum within block + b_eff
  b_eff = bias with bias[0] doubled at position 0.

Key identity: per 8-token block, Out = L @ X @ W^T + b_eff where L is the
strictly-lower-triangular 8x8 ones matrix.  Both factors are matmuls:

  mmA (cumsum fused into transpose):  Z[e, s'] = sum_s X[s, e] * LT[s, s']
      where LT[s, s'] = 1 iff s, s' in same 8-block and s < s'
      (LT is the 128x128 block-diagonal strictly-upper ones matrix).
      lhsT = X chunk [s(128 part), e(128)] -- natural DMA layout!
      rhs  = LT [s(128 part), s'(128)]
      => Z = (L@X)^T lands e-on-partitions, exactly what mmB needs.

  mmB (projection): O[s', t] = sum_e Z[e, s'] * WT[e, t], K=E=256 via two
      accumulating matmuls.  O is token-on-partition => contiguous store.

  bias: per-partition bias add fused into the PSUM->SBUF eviction
      (ScalarE activation Identity with bias AP [128,1], b_eff[p % 8]).

Sharding: data-parallel over N=8 batches, one batch per NeuronCore.
"""

import os

import numpy as np

import concourse.bass as bass
import concourse.mybir as mybir
import concourse.tile as tile
from concourse import bacc
from concourse.bass_utils import run_bass_kernel_spmd

N, S, E, BS = 8, 8192, 256, 8
P = 128                 # tokens per tile / partitions
PACK = 8                # token-tiles per DMA chunk
CHUNK = P * PACK        # tokens per chunk
NCHUNK = S // CHUNK
NCORES = 8

# Precision mode:
#   f32   : everything float32 (exact, PE-bound ~4 cyc/row)
#   f32r  : matmuls in float32r (single-pass fp32, reduced internal precision)
#   bf16  : matmul operands cast to bf16 (fp32 accumulation)
MODE = os.environ.get("BCONV_MODE", "f32r")

_cache = {}


def _build_nc(mode: str):
    f32 = mybir.dt.float32
    f32r = mybir.dt.float32r
    bf16 = mybir.dt.bfloat16

    # dtype of SBUF tiles feeding mmA (x, lt) and mmB (z, wt)
    if mode == "f32":
        a_dt, b_dt = f32, f32
    elif mode == "f32r":
        a_dt, b_dt = f32r, f32r
    elif mode == "bf16":
        a_dt, b_dt = bf16, bf16
    else:
        raise ValueError(mode)

    nc = bacc.Bacc(
        "TRN2", target_bir_lowering=False, debug=False, num_devices=NCORES)
    # x dram dtype matches the tile dtype for f32r (bit-identical, no cast);
    # stays f32 for bf16 (GpSimd casts on-chip so loads stay on HWDGE).
    x_dram_dt = a_dt if a_dt == f32r else f32
    x = nc.dram_tensor("x", [S, E], x_dram_dt, kind="ExternalInput")
    lt = nc.dram_tensor("lt", [P, P], a_dt, kind="ExternalInput")
    wt = nc.dram_tensor("wt", [P, 2 * E], b_dt, kind="ExternalInput")
    beff = nc.dram_tensor("beff", [P, 1], f32, kind="ExternalInput")
    y = nc.dram_tensor("y", [S, E], f32, kind="ExternalOutput")

    x_cast = a_dt == bf16
    ident = mybir.ActivationFunctionType.Identity
    G = 2                    # j-tiles per PSUM group (one 2KB bank each)

    with tile.TileContext(nc) as tc:
        with (
            tc.tile_pool(name="const", bufs=1) as constp,
            tc.tile_pool(name="xin", bufs=3) as xin,
            tc.tile_pool(name="xbf", bufs=3) as xbf,
            tc.tile_pool(name="zsb", bufs=4) as zsbp,
            tc.tile_pool(name="yout", bufs=3) as yout,
            tc.tile_pool(name="zps", bufs=3, space="PSUM") as zps,
            tc.tile_pool(name="ops", bufs=3, space="PSUM") as ops,
        ):
            lt_sb = constp.tile([P, P], a_dt)
            nc.sync.dma_start(lt_sb[:], lt[:, :])
            wt_sb = constp.tile([P, 2 * E], b_dt)
            nc.sync.dma_start(wt_sb[:], wt[:, :])
            beff_sb = constp.tile([P, 1], f32)
            nc.sync.dma_start(beff_sb[:], beff[:, :])

            for c in range(NCHUNK):
                xsl = x[c * CHUNK:(c + 1) * CHUNK, :].rearrange(
                    "(i p) e -> p i e", p=P)
                xt = xin.tile([P, PACK * E], x_dram_dt)
                nc.sync.dma_start(xt[:], xsl)
                if x_cast:
                    xb = xbf.tile([P, PACK * E], bf16)
                    nc.gpsimd.tensor_copy(xb[:], xt[:])
                else:
                    xb = xt

                ot = yout.tile([P, PACK * E], f32)
                for h in range(PACK // G):
                    # mmA group: 2*G chunks of (L@X)^T into one PSUM bank
                    zp = zps.tile([P, 2 * G * P], f32)
                    for jj in range(G):
                        j = G * h + jj
                        for k in range(2):
                            m = 2 * jj + k
                            nc.tensor.matmul(
                                zp[:, m * P:(m + 1) * P],
                                xb[:, j * E + k * P: j * E + (k + 1) * P],
                                lt_sb[:],
                                start=True, stop=True,
                            )
                    zt = zsbp.tile([P, 2 * G * P], b_dt)
                    nc.vector.tensor_copy(zt[:], zp[:])
                    # mmB group: G projected j-tiles into one PSUM bank
                    op = ops.tile([P, G * E], f32)
                    for jj in range(G):
                        nc.tensor.matmul(
                            op[:, jj * E:(jj + 1) * E],
                            zt[:, 2 * jj * P:(2 * jj + 1) * P],
                            wt_sb[:, 0:E],
                            start=True, stop=False)
                        nc.tensor.matmul(
                            op[:, jj * E:(jj + 1) * E],
                            zt[:, (2 * jj + 1) * P:(2 * jj + 2) * P],
                            wt_sb[:, E:2 * E],
                            start=False, stop=True)
                    nc.scalar.activation(
                        ot[:, h * G * E:(h + 1) * G * E], op[:], ident,
                        bias=beff_sb[:])

                ysl = y[c * CHUNK:(c + 1) * CHUNK, :].rearrange(
                    "(i p) e -> p i e", p=P)
                nc.sync.dma_start(ysl, ot[:])
    nc.compile()
    return nc


def _np_dt(dt):
    import ml_dtypes
    return {"bfloat16": ml_dtypes.bfloat16}.get(dt, np.float32)


def _host_consts(W: np.ndarray, b: np.ndarray, mode: str):
    if mode == "f32" or mode == "f32r":
        a_np, b_np = np.float32, np.float32
    elif mode == "bf16":
        a_np, b_np = _np_dt("bfloat16"), _np_dt("bfloat16")
    else:
        raise ValueError(mode)

    idx = np.arange(P)
    blk = idx // BS
    LT = ((blk[:, None] == blk[None, :]) & (idx[:, None] < idx[None, :]))
    LT = np.ascontiguousarray(LT.astype(a_np))

    WT = W.T.astype(np.float32)                      # [E, T]
    WTP = np.concatenate([WT[0:P, :], WT[P:2 * P, :]], axis=1)  # [128, 512]
    WTP = np.ascontiguousarray(WTP.astype(b_np))

    be = b.astype(np.float64).copy()
    be[0] += be[0]
    BEFF = np.ascontiguousarray(
        be[idx % BS].astype(np.float32).reshape(P, 1))
    return LT, WTP, BEFF


def kernel(seq_vector, W, bias):
    mode = MODE
    x = np.ascontiguousarray(np.asarray(seq_vector, dtype=np.float32))
    W = np.asarray(W, dtype=np.float32)
    b = np.asarray(bias, dtype=np.float32)
    assert x.shape == (N, S, E)

    LT, WTP, BEFF = _host_consts(W, b, mode)

    if mode not in _cache:
        _cache[mode] = _build_nc(mode)
    nc = _cache[mode]

    in_maps = [
        {"x": x[i], "lt": LT, "wt": WTP, "beff": BEFF}
        for i in range(NCORES)
    ]
    res = run_bass_kernel_spmd(nc, in_maps, core_ids=list(range(NCORES)))
    out = np.stack([r["y"] for r in res.results], axis=0)
    return out.reshape(N, S, E)


# revision 12
# speedup vs baseline: 1.0773x; 1.0773x over previous
"""Trainium2 Bass kernel for nn_BlockConvolutionLean.

Computation (see reference):
  features = einsum('nse,te->nst', seq_vector, W)        # 1x1 conv
  blocks of BS=8 along S; out = exclusive-cumsum within block + b_eff
  b_eff = bias with bias[0] doubled at position 0.

Key identity: per 8-token block, Out = L @ X @ W^T + b_eff where L is the
strictly-lower-triangular 8x8 ones matrix.  Both factors are matmuls:

  mmA (cumsum fused into transpose):  Z[e, s'] = sum_s X[s, e] * LT[s, s']
      where LT[s, s'] = 1 iff s, s' in same 8-block and s < s'
      (LT is the 128x128 block-diagonal strictly-upper ones matrix).
      lhsT = X chunk [s(128 part), e(128)] -- natural DMA layout!
      rhs  = LT [s(128 part), s'(128)]
      => Z = (L@X)^T lands e-on-partitions, exactly what mmB needs.

  mmB (projection): O[s', t] = sum_e Z[e, s'] * WT[e, t], K=E=256 via two
      accumulating matmuls.  O is token-on-partition => contiguous store.

  bias: per-partition bias add fused into the PSUM->SBUF eviction
      (ScalarE activation Identity with bias AP [128,1], b_eff[p % 8]).

Sharding: data-parallel over N=8 batches, one batch per NeuronCore.
"""

import os

import numpy as np

import concourse.bass as bass
import concourse.mybir as mybir
import concourse.tile as tile
from concourse import bacc
from concourse.bass_utils import run_bass_kernel_spmd

N, S, E, BS = 8, 8192, 256, 8
P = 128                 # tokens per tile / partitions
PACK = 8                # token-tiles per DMA chunk
CHUNK = P * PACK        # tokens per chunk
NCHUNK = S // CHUNK
NCORES = 8

# Precision mode:
#   f32   : everything float32 (exact, PE-bound ~4 cyc/row)
#   f32r  : matmuls in float32r (single-pass fp32, reduced internal precision)
#   bf16  : matmul operands cast to bf16 (fp32 accumulation)
MODE = os.environ.get("BCONV_MODE", "f32r")

_cache = {}


def _build_nc(mode: str):
    f32 = mybir.dt.float32
    f32r = mybir.dt.float32r
    bf16 = mybir.dt.bfloat16

    # dtype of SBUF tiles feeding mmA (x, lt) and mmB (z, wt)
    if mode == "f32":
        a_dt, b_dt = f32, f32
    elif mode == "f32r":
        a_dt, b_dt = f32r, f32r
    elif mode == "bf16":
        a_dt, b_dt = bf16, bf16
    else:
        raise ValueError(mode)

    nc = bacc.Bacc(
        "TRN2", target_bir_lowering=False, debug=False, num_devices=NCORES)
    # x dram dtype matches the tile dtype for f32r (bit-identical, no cast);
    # stays f32 for bf16 (GpSimd casts on-chip so loads stay on HWDGE).
    x_dram_dt = a_dt if a_dt == f32r else f32
    x = nc.dram_tensor("x", [S, E], x_dram_dt, kind="ExternalInput")
    lt = nc.dram_tensor("lt", [P, P], a_dt, kind="ExternalInput")
    wt = nc.dram_tensor("wt", [P, 2 * E], b_dt, kind="ExternalInput")
    beff = nc.dram_tensor("beff", [P, 1], f32, kind="ExternalInput")
    y = nc.dram_tensor("y", [S, E], f32, kind="ExternalOutput")

    x_cast = a_dt == bf16
    ident = mybir.ActivationFunctionType.Identity
    G = 2                    # j-tiles per PSUM group (one 2KB bank each)

    with tile.TileContext(nc) as tc:
        with (
            tc.tile_pool(name="const", bufs=1) as constp,
            tc.tile_pool(name="xin", bufs=3) as xin,
            tc.tile_pool(name="zsb", bufs=4) as zsbp,
            tc.tile_pool(name="yout", bufs=3) as yout,
            tc.tile_pool(name="zps", bufs=3, space="PSUM") as zps,
            tc.tile_pool(name="ops", bufs=3, space="PSUM") as ops,
        ):
            lt_sb = constp.tile([P, P], a_dt)
            nc.sync.dma_start(lt_sb[:], lt[:, :])
            wt_sb = constp.tile([P, 2 * E], b_dt)
            nc.sync.dma_start(wt_sb[:], wt[:, :])
            beff_sb = constp.tile([P, 1], f32)
            nc.sync.dma_start(beff_sb[:], beff[:, :])

            for c in range(NCHUNK):
                xsl = x[c * CHUNK:(c + 1) * CHUNK, :].rearrange(
                    "(i p) e -> p i e", p=P)
                xb = xin.tile([P, PACK * E], a_dt)
                if x_cast:
                    # SWDGE casts f32->bf16 in flight; keeps HWDGE free
                    # for output stores.
                    nc.gpsimd.dma_start(xb[:], xsl)
                else:
                    nc.sync.dma_start(xb[:], xsl)

                ot = yout.tile([P, PACK * E], f32)
                for h in range(PACK // G):
                    # mmA group: 2*G chunks of (L@X)^T into one PSUM bank
                    zp = zps.tile([P, 2 * G * P], f32)
                    for jj in range(G):
                        j = G * h + jj
                        for k in range(2):
                            m = 2 * jj + k
                            nc.tensor.matmul(
                                zp[:, m * P:(m + 1) * P],
                                xb[:, j * E + k * P: j * E + (k + 1) * P],
                                lt_sb[:],
                                start=True, stop=True,
                            )
                    zt = zsbp.tile([P, 2 * G * P], b_dt)
                    nc.vector.tensor_copy(zt[:], zp[:])
                    # mmB group: G projected j-tiles into one PSUM bank
                    op = ops.tile([P, G * E], f32)
                    for jj in range(G):
                        nc.tensor.matmul(
                            op[:, jj * E:(jj + 1) * E],
                            zt[:, 2 * jj * P:(2 * jj + 1) * P],
                            wt_sb[:, 0:E],
                            start=True, stop=False)
                        nc.tensor.matmul(
                            op[:, jj * E:(jj + 1) * E],
                            zt[:, (2 * jj + 1) * P:(2 * jj + 2) * P],
                            wt_sb[:, E:2 * E],
                            start=False, stop=True)
                    nc.scalar.activation(
                        ot[:, h * G * E:(h + 1) * G * E], op[:], ident,
                        bias=beff_sb[:])

                ysl = y[c * CHUNK:(c + 1) * CHUNK, :].rearrange(
                    "(i p) e -> p i e", p=P)
                nc.sync.dma_start(ysl, ot[:])
    nc.compile()
    return nc


def _np_dt(dt):
    import ml_dtypes
    return {"bfloat16": ml_dtypes.bfloat16}.get(dt, np.float32)


def _host_consts(W: np.ndarray, b: np.ndarray, mode: str):
    if mode == "f32" or mode == "f32r":
        a_np, b_np = np.float32, np.float32
    elif mode == "bf16":
        a_np, b_np = _np_dt("bfloat16"), _np_dt("bfloat16")
    else:
        raise ValueError(mode)

    idx = np.arange(P)
    blk = idx // BS
    LT = ((blk[:, None] == blk[None, :]) & (idx[:, None] < idx[None, :]))
    LT = np.ascontiguousarray(LT.astype(a_np))

    WT = W.T.astype(np.float32)                      # [E, T]
    WTP = np.concatenate([WT[0:P, :], WT[P:2 * P, :]], axis=1)  # [128, 512]
    WTP = np.ascontiguousarray(WTP.astype(b_np))

    be = b.astype(np.float64).copy()
    be[0] += be[0]
    BEFF = np.ascontiguousarray(
        be[idx % BS].astype(np.float32).reshape(P, 1))
    return LT, WTP, BEFF


def kernel(seq_vector, W, bias):
    mode = MODE
    x = np.ascontiguousarray(np.asarray(seq_vector, dtype=np.float32))
    W = np.asarray(W, dtype=np.float32)
    b = np.asarray(bias, dtype=np.float32)
    assert x.shape == (N, S, E)

    LT, WTP, BEFF = _host_consts(W, b, mode)

    if mode not in _cache:
        _cache[mode] = _build_nc(mode)
    nc = _cache[mode]

    in_maps = [
        {"x": x[i], "lt": LT, "wt": WTP, "beff": BEFF}
        for i in range(NCORES)
    ]
    res = run_bass_kernel_spmd(nc, in_maps, core_ids=list(range(NCORES)))
    out = np.stack([r["y"] for r in res.results], axis=0)
    return out.reshape(N, S, E)


# revision 13
# speedup vs baseline: 1.2678x; 1.1769x over previous
"""Trainium2 Bass kernel for nn_BlockConvolutionLean.

Computation (see reference):
  features = einsum('nse,te->nst', seq_vector, W)        # 1x1 conv
  blocks of BS=8 along S; out = exclusive-cumsum within block + b_eff
  b_eff = bias with bias[0] doubled at position 0.

Key identity: per 8-token block, Out = L @ X @ W^T + b_eff where L is the
strictly-lower-triangular 8x8 ones matrix.  Both factors are matmuls:

  mmA (cumsum fused into transpose):  Z[e, s'] = sum_s X[s, e] * LT[s, s']
      where LT[s, s'] = 1 iff s, s' in same 8-block and s < s'
      (LT is the 128x128 block-diagonal strictly-upper ones matrix).
      lhsT = X chunk [s(128 part), e(128)] -- natural DMA layout!
      rhs  = LT [s(128 part), s'(128)]
      => Z = (L@X)^T lands e-on-partitions, exactly what mmB needs.

  mmB (projection): O[s', t] = sum_e Z[e, s'] * WT[e, t], K=E=256 via two
      accumulating matmuls.  O is token-on-partition => contiguous store.

  bias: per-partition bias add fused into the PSUM->SBUF eviction
      (ScalarE activation Identity with bias AP [128,1], b_eff[p % 8]).

Sharding: data-parallel over N=8 batches, one batch per NeuronCore.

Precision modes (BCONV_MODE):
  f32   : everything float32 (PE-bound: fp32 matmul = 4 cyc/row)
  f32r  : float32r matmuls (single-pass fp32; LDWEIGHTS-bound)
  bf16  : host stages x as bf16 (halves input HBM traffic); all matmul
          operands bf16, fp32 PSUM accumulation
"""

import os

import numpy as np

import concourse.mybir as mybir
import concourse.tile as tile
from concourse import bacc
from concourse.bass_utils import run_bass_kernel_spmd

N, S, E, BS = 8, 8192, 256, 8
P = 128                 # tokens per tile / partitions
NCORES = 8
G = 2                   # j-tiles per PSUM group (one 2KB bank each)
# chunk schedule in 128-token tiles: small first chunks cut the
# time-to-first-matmul; 8-tile (1 MiB f32 / 512 KiB bf16) steady state
SCHED = [2, 2, 4] + [8] * 7
assert sum(SCHED) == S // P and all(c % G == 0 for c in SCHED)

MODE = os.environ.get("BCONV_MODE", "bf16")

_cache = {}


def _build_nc(mode: str):
    f32 = mybir.dt.float32
    f32r = mybir.dt.float32r
    bf16 = mybir.dt.bfloat16

    # dtype of SBUF tiles feeding mmA (x, lt) and mmB (z, wt)
    if mode == "f32":
        a_dt, b_dt = f32, f32
    elif mode == "f32r":
        a_dt, b_dt = f32r, f32r
    elif mode == "bf16":
        a_dt, b_dt = bf16, bf16
    else:
        raise ValueError(mode)

    nc = bacc.Bacc(
        "TRN2", target_bir_lowering=False, debug=False, num_devices=NCORES)
    # x arrives pre-staged in the matmul dtype (host casts for bf16;
    # f32r is bit-identical to f32) -> every load is a plain HWDGE DMA.
    x = nc.dram_tensor("x", [S, E], a_dt, kind="ExternalInput")
    lt = nc.dram_tensor("lt", [P, P], a_dt, kind="ExternalInput")
    wt = nc.dram_tensor("wt", [P, 2 * E], b_dt, kind="ExternalInput")
    beff = nc.dram_tensor("beff", [P, 1], f32, kind="ExternalInput")
    y = nc.dram_tensor("y", [S, E], f32, kind="ExternalOutput")

    ident = mybir.ActivationFunctionType.Identity
    max_pack = max(SCHED)

    with tile.TileContext(nc) as tc:
        with (
            tc.tile_pool(name="const", bufs=1) as constp,
            tc.tile_pool(name="xin", bufs=4) as xin,
            tc.tile_pool(name="zsb", bufs=4) as zsbp,
            tc.tile_pool(name="yout", bufs=6) as yout,
            tc.tile_pool(name="zps", bufs=3, space="PSUM") as zps,
            tc.tile_pool(name="ops", bufs=3, space="PSUM") as ops,
        ):
            lt_sb = constp.tile([P, P], a_dt)
            nc.sync.dma_start(lt_sb[:], lt[:, :])
            wt_sb = constp.tile([P, 2 * E], b_dt)
            nc.sync.dma_start(wt_sb[:], wt[:, :])
            beff_sb = constp.tile([P, 1], f32)
            nc.sync.dma_start(beff_sb[:], beff[:, :])

            tile0 = 0
            for pack in SCHED:
                chunk = pack * P
                xsl = x[tile0 * P:tile0 * P + chunk, :].rearrange(
                    "(i p) e -> p i e", p=P)
                xb = xin.tile([P, max_pack * E], a_dt, tag="xb")
                nc.sync.dma_start(xb[:, 0:pack * E], xsl)

                for h in range(pack // G):
                    # mmA group: 2*G tiles of (L@X)^T into one PSUM bank
                    zp = zps.tile([P, 2 * G * P], f32)
                    for jj in range(G):
                        j = G * h + jj
                        for k in range(2):
                            m = 2 * jj + k
                            nc.tensor.matmul(
                                zp[:, m * P:(m + 1) * P],
                                xb[:, j * E + k * P: j * E + (k + 1) * P],
                                lt_sb[:],
                                start=True, stop=True,
                            )
                    zt = zsbp.tile([P, 2 * G * P], b_dt)
                    nc.vector.tensor_copy(zt[:], zp[:])
                    # mmB group: G projected j-tiles into one PSUM bank
                    op = ops.tile([P, G * E], f32)
                    for jj in range(G):
                        nc.tensor.matmul(
                            op[:, jj * E:(jj + 1) * E],
                            zt[:, 2 * jj * P:(2 * jj + 1) * P],
                            wt_sb[:, 0:E],
                            start=True, stop=False)
                        nc.tensor.matmul(
                            op[:, jj * E:(jj + 1) * E],
                            zt[:, (2 * jj + 1) * P:(2 * jj + 2) * P],
                            wt_sb[:, E:2 * E],
                            start=False, stop=True)
                    # eviction with fused per-partition bias add
                    ot = yout.tile([P, G * E], f32)
                    nc.scalar.activation(ot[:], op[:], ident, bias=beff_sb[:])
                    ysl = y[(tile0 + h * G) * P:(tile0 + (h + 1) * G) * P,
                            :].rearrange("(i p) e -> p i e", p=P)
                    nc.sync.dma_start(ysl, ot[:])
                tile0 += pack
    nc.compile()
    return nc


def _np_dt(name):
    import ml_dtypes
    return {"bfloat16": ml_dtypes.bfloat16}.get(name, np.float32)


def _host_consts(W: np.ndarray, b: np.ndarray, mode: str):
    if mode in ("f32", "f32r"):
        a_np, b_np = np.float32, np.float32
    elif mode == "bf16":
        a_np, b_np = _np_dt("bfloat16"), _np_dt("bfloat16")
    else:
        raise ValueError(mode)

    idx = np.arange(P)
    blk = idx // BS
    LT = ((blk[:, None] == blk[None, :]) & (idx[:, None] < idx[None, :]))
    LT = np.ascontiguousarray(LT.astype(a_np))

    WT = W.T.astype(np.float32)                      # [E, T]
    WTP = np.concatenate([WT[0:P, :], WT[P:2 * P, :]], axis=1)  # [128, 512]
    WTP = np.ascontiguousarray(WTP.astype(b_np))

    be = b.astype(np.float64).copy()
    be[0] += be[0]
    BEFF = np.ascontiguousarray(
        be[idx % BS].astype(np.float32).reshape(P, 1))
    return LT, WTP, BEFF, a_np


def kernel(seq_vector, W, bias):
    mode = MODE
    x = np.asarray(seq_vector, dtype=np.float32)
    W = np.asarray(W, dtype=np.float32)
    b = np.asarray(bias, dtype=np.float32)
    assert x.shape == (N, S, E)

    LT, WTP, BEFF, a_np = _host_consts(W, b, mode)
    xs = np.ascontiguousarray(x.astype(a_np))

    if mode not in _cache:
        _cache[mode] = _build_nc(mode)
    nc = _cache[mode]

    in_maps = [
        {"x": xs[i], "lt": LT, "wt": WTP, "beff": BEFF}
        for i in range(NCORES)
    ]
    res = run_bass_kernel_spmd(nc, in_maps, core_ids=list(range(NCORES)))
    out = np.stack([r["y"] for r in res.results], axis=0)
    return out.reshape(N, S, E)


# revision 15
# speedup vs baseline: 1.3696x; 1.0803x over previous
"""Trainium2 Bass kernel for nn_BlockConvolutionLean.

Computation (see reference):
  features = einsum('nse,te->nst', seq_vector, W)        # 1x1 conv
  blocks of BS=8 along S; out = exclusive-cumsum within block + b_eff
  b_eff = bias with bias[0] doubled at position 0.

Key identity: per 8-token block, Out = L @ X @ W^T + b_eff where L is the
strictly-lower-triangular 8x8 ones matrix.  Both factors are matmuls:

  mmA (cumsum fused into transpose):  Z[e, s'] = sum_s X[s, e] * LT[s, s']
      where LT[s, s'] = 1 iff s, s' in same 8-block and s < s'
      (LT is the 128x128 block-diagonal strictly-upper ones matrix).
      lhsT = X chunk [s(128 part), e(128)] -- natural DMA layout!
      rhs  = LT [s(128 part), s'(128)]
      => Z = (L@X)^T lands e-on-partitions, exactly what mmB needs.

  mmB (projection): O[s', t] = sum_e Z[e, s'] * WT[e, t], K=E=256 via two
      accumulating matmuls.  O is token-on-partition => contiguous store.

  bias: per-partition bias add fused into the PSUM->SBUF eviction
      (ScalarE activation Identity with bias AP [128,1], b_eff[p % 8]).

Sharding: data-parallel over N=8 batches, one batch per NeuronCore.

Precision modes (BCONV_MODE):
  f32   : everything float32 (PE-bound: fp32 matmul = 4 cyc/row)
  f32r  : float32r matmuls (single-pass fp32; LDWEIGHTS-bound)
  bf16  : host stages x as bf16 (halves input HBM traffic); all matmul
          operands bf16, fp32 PSUM accumulation
"""

import os

import numpy as np

import concourse.mybir as mybir
import concourse.tile as tile
from concourse import bacc
from concourse.bass_utils import run_bass_kernel_spmd

N, S, E, BS = 8, 8192, 256, 8
P = 128                 # tokens per tile / partitions
NCORES = 8
G = 2                   # j-tiles per PSUM group (one 2KB bank each)
# chunk schedule in 128-token tiles: small chunks at both ends cut the
# time-to-first-matmul and the last-store tail; 8-tile steady state
SCHED = [2, 2, 4] + [8] * 6 + [4, 2, 2]
assert sum(SCHED) == S // P and all(c % G == 0 for c in SCHED)

MODE = os.environ.get("BCONV_MODE", "bf16")

_cache = {}


def _build_nc(mode: str):
    f32 = mybir.dt.float32
    f32r = mybir.dt.float32r
    bf16 = mybir.dt.bfloat16

    # dtype of SBUF tiles feeding mmA (x, lt) and mmB (z, wt)
    if mode == "f32":
        a_dt, b_dt = f32, f32
    elif mode == "f32r":
        a_dt, b_dt = f32r, f32r
    elif mode == "bf16":
        a_dt, b_dt = bf16, bf16
    else:
        raise ValueError(mode)

    nc = bacc.Bacc(
        "TRN2", target_bir_lowering=False, debug=False, num_devices=NCORES)
    # x arrives pre-staged in the matmul dtype (host casts for bf16;
    # f32r is bit-identical to f32) -> every load is a plain HWDGE DMA.
    x = nc.dram_tensor("x", [S, E], a_dt, kind="ExternalInput")
    lt = nc.dram_tensor("lt", [P, P], a_dt, kind="ExternalInput")
    wt = nc.dram_tensor("wt", [P, 2 * E], b_dt, kind="ExternalInput")
    beff = nc.dram_tensor("beff", [P, 1], f32, kind="ExternalInput")
    y = nc.dram_tensor("y", [S, E], f32, kind="ExternalOutput")

    ident = mybir.ActivationFunctionType.Identity
    max_pack = max(SCHED)

    with tile.TileContext(nc) as tc:
        with (
            tc.tile_pool(name="const", bufs=1) as constp,
            tc.tile_pool(name="xin", bufs=4) as xin,
            tc.tile_pool(name="zsb", bufs=4) as zsbp,
            tc.tile_pool(name="yout", bufs=6) as yout,
            tc.tile_pool(name="zps", bufs=3, space="PSUM") as zps,
            tc.tile_pool(name="ops", bufs=3, space="PSUM") as ops,
        ):
            lt_sb = constp.tile([P, P], a_dt)
            nc.sync.dma_start(lt_sb[:], lt[:, :])
            wt_sb = constp.tile([P, 2 * E], b_dt)
            nc.sync.dma_start(wt_sb[:], wt[:, :])
            beff_sb = constp.tile([P, 1], f32)
            nc.sync.dma_start(beff_sb[:], beff[:, :])

            tile0 = 0
            for pack in SCHED:
                chunk = pack * P
                xsl = x[tile0 * P:tile0 * P + chunk, :].rearrange(
                    "(i p) e -> p i e", p=P)
                xb = xin.tile([P, max_pack * E], a_dt, tag="xb")
                nc.sync.dma_start(xb[:, 0:pack * E], xsl)

                for h in range(pack // G):
                    # mmA group: 2*G tiles of (L@X)^T into one PSUM bank
                    zp = zps.tile([P, 2 * G * P], f32)
                    for jj in range(G):
                        j = G * h + jj
                        for k in range(2):
                            m = 2 * jj + k
                            nc.tensor.matmul(
                                zp[:, m * P:(m + 1) * P],
                                xb[:, j * E + k * P: j * E + (k + 1) * P],
                                lt_sb[:],
                                start=True, stop=True,
                            )
                    zt = zsbp.tile([P, 2 * G * P], b_dt)
                    nc.vector.tensor_copy(zt[:], zp[:])
                    # mmB group: G projected j-tiles into one PSUM bank
                    op = ops.tile([P, G * E], f32)
                    for jj in range(G):
                        nc.tensor.matmul(
                            op[:, jj * E:(jj + 1) * E],
                            zt[:, 2 * jj * P:(2 * jj + 1) * P],
                            wt_sb[:, 0:E],
                            start=True, stop=False)
                        nc.tensor.matmul(
                            op[:, jj * E:(jj + 1) * E],
                            zt[:, (2 * jj + 1) * P:(2 * jj + 2) * P],
                            wt_sb[:, E:2 * E],
                            start=False, stop=True)
                    # eviction with fused per-partition bias add
                    ot = yout.tile([P, G * E], f32)
                    nc.scalar.activation(ot[:], op[:], ident, bias=beff_sb[:])
                    ysl = y[(tile0 + h * G) * P:(tile0 + (h + 1) * G) * P,
                            :].rearrange("(i p) e -> p i e", p=P)
                    # stores ride SWDGE so input prefetch never queues
                    # behind them on the HWDGE FIFO
                    nc.gpsimd.dma_start(ysl, ot[:])
                tile0 += pack
    nc.compile()
    return nc


def _np_dt(name):
    import ml_dtypes
    return {"bfloat16": ml_dtypes.bfloat16}.get(name, np.float32)


def _host_consts(W: np.ndarray, b: np.ndarray, mode: str):
    if mode in ("f32", "f32r"):
        a_np, b_np = np.float32, np.float32
    elif mode == "bf16":
        a_np, b_np = _np_dt("bfloat16"), _np_dt("bfloat16")
    else:
        raise ValueError(mode)

    idx = np.arange(P)
    blk = idx // BS
    LT = ((blk[:, None] == blk[None, :]) & (idx[:, None] < idx[None, :]))
    LT = np.ascontiguousarray(LT.astype(a_np))

    WT = W.T.astype(np.float32)                      # [E, T]
    WTP = np.concatenate([WT[0:P, :], WT[P:2 * P, :]], axis=1)  # [128, 512]
    WTP = np.ascontiguousarray(WTP.astype(b_np))

    be = b.astype(np.float64).copy()
    be[0] += be[0]
    BEFF = np.ascontiguousarray(
        be[idx % BS].astype(np.float32).reshape(P, 1))
    return LT, WTP, BEFF, a_np


def kernel(seq_vector, W, bias):
    mode = MODE
    x = np.asarray(seq_vector, dtype=np.float32)
    W = np.asarray(W, dtype=np.float32)
    b = np.asarray(bias, dtype=np.float32)
    assert x.shape == (N, S, E)

    LT, WTP, BEFF, a_np = _host_consts(W, b, mode)
    xs = np.ascontiguousarray(x.astype(a_np))

    if mode not in _cache:
        _cache[mode] = _build_nc(mode)
    nc = _cache[mode]

    in_maps = [
        {"x": xs[i], "lt": LT, "wt": WTP, "beff": BEFF}
        for i in range(NCORES)
    ]
    res = run_bass_kernel_spmd(nc, in_maps, core_ids=list(range(NCORES)))
    out = np.stack([r["y"] for r in res.results], axis=0)
    return out.reshape(N, S, E)


# revision 16
# speedup vs baseline: 1.4591x; 1.0653x over previous
"""Trainium2 Bass kernel for nn_BlockConvolutionLean.

Computation (see reference):
  features = einsum('nse,te->nst', seq_vector, W)        # 1x1 conv
  blocks of BS=8 along S; out = exclusive-cumsum within block + b_eff
  b_eff = bias with bias[0] doubled at position 0.

Key identity: per 8-token block, Out = L @ X @ W^T + b_eff where L is the
strictly-lower-triangular 8x8 ones matrix.  Both factors are matmuls:

  mmA (cumsum fused into transpose):  Z[e, s'] = sum_s X[s, e] * LT[s, s']
      where LT[s, s'] = 1 iff s, s' in same 8-block and s < s'
      (LT is the 128x128 block-diagonal strictly-upper ones matrix).
      lhsT = X chunk [s(128 part), e(128)] -- natural DMA layout!
      rhs  = LT [s(128 part), s'(128)]
      => Z = (L@X)^T lands e-on-partitions, exactly what mmB needs.

  mmB (projection): O[s', t] = sum_e Z[e, s'] * WT[e, t], K=E=256 via two
      accumulating matmuls.  O is token-on-partition => contiguous store.

  bias: per-partition bias add fused into the PSUM->SBUF eviction
      (ScalarE activation Identity with bias AP [128,1], b_eff[p % 8]).

Sharding: data-parallel over N=8 batches, one batch per NeuronCore.

Precision modes (BCONV_MODE):
  f32   : everything float32 (PE-bound: fp32 matmul = 4 cyc/row)
  f32r  : float32r matmuls (single-pass fp32; LDWEIGHTS-bound)
  bf16  : host stages x as bf16 (halves input HBM traffic); all matmul
          operands bf16, fp32 PSUM accumulation
"""

import os

import numpy as np

import concourse.mybir as mybir
import concourse.tile as tile
from concourse import bacc
from concourse.bass_utils import run_bass_kernel_spmd

N, S, E, BS = 8, 8192, 256, 8
P = 128                 # tokens per tile / partitions
NCORES = 8
G = 2                   # j-tiles per PSUM group (one 2KB bank each)
# chunk schedule in 128-token tiles: small chunks at both ends cut the
# time-to-first-matmul and the last-store tail; 8-tile steady state
SCHED = [2, 2, 4] + [8] * 6 + [4, 2, 2]
assert sum(SCHED) == S // P and all(c % G == 0 for c in SCHED)

MODE = os.environ.get("BCONV_MODE", "bf16")

_cache = {}


def _build_nc(mode: str):
    f32 = mybir.dt.float32
    f32r = mybir.dt.float32r
    bf16 = mybir.dt.bfloat16

    # dtype of SBUF tiles feeding mmA (x, lt) and mmB (z, wt)
    if mode == "f32":
        a_dt, b_dt = f32, f32
    elif mode == "f32r":
        a_dt, b_dt = f32r, f32r
    elif mode == "bf16":
        a_dt, b_dt = bf16, bf16
    else:
        raise ValueError(mode)

    nc = bacc.Bacc(
        "TRN2", target_bir_lowering=False, debug=False, num_devices=NCORES)
    # x arrives pre-staged in the matmul dtype (host casts for bf16;
    # f32r is bit-identical to f32) -> every load is a plain HWDGE DMA.
    x = nc.dram_tensor("x", [S, E], a_dt, kind="ExternalInput")
    lt = nc.dram_tensor("lt", [P, P], a_dt, kind="ExternalInput")
    wt = nc.dram_tensor("wt", [P, 2 * E], b_dt, kind="ExternalInput")
    beff = nc.dram_tensor("beff", [P, 1], f32, kind="ExternalInput")
    y = nc.dram_tensor("y", [S, E], f32, kind="ExternalOutput")

    ident = mybir.ActivationFunctionType.Identity
    max_pack = max(SCHED)

    with tile.TileContext(nc) as tc:
        with (
            tc.tile_pool(name="const", bufs=1) as constp,
            tc.tile_pool(name="xin", bufs=6) as xin,
            tc.tile_pool(name="zsb", bufs=6) as zsbp,
            tc.tile_pool(name="yout", bufs=8) as yout,
            tc.tile_pool(name="zps", bufs=3, space="PSUM") as zps,
            tc.tile_pool(name="ops", bufs=3, space="PSUM") as ops,
            tc.tile_pool(name="wps", bufs=1, space="PSUM") as wps,
        ):
            lt_sb = constp.tile([P, P], a_dt)
            nc.sync.dma_start(lt_sb[:], lt[:, :])
            wt_sb = constp.tile([P, 2 * E], b_dt)
            nc.sync.dma_start(wt_sb[:], wt[:, :])
            beff_sb = constp.tile([P, 1], f32)
            nc.sync.dma_start(beff_sb[:], beff[:, :])

            # HAM warmup: keep the PE busy while the first input chunks
            # stream in, so real matmuls start at 2.4 GHz (K=8/8).
            wset = constp.tile([P, P], a_dt)
            nc.gpsimd.memset(wset[:], 0.0)
            wp = wps.tile([P, P], f32)
            for _ in range(28):
                nc.tensor.matmul(wp[:], wset[:], wset[:], start=True, stop=True)

            tile0 = 0
            for pack in SCHED:
                chunk = pack * P
                xsl = x[tile0 * P:tile0 * P + chunk, :].rearrange(
                    "(i p) e -> p i e", p=P)
                xb = xin.tile([P, max_pack * E], a_dt, tag="xb")
                nc.sync.dma_start(xb[:, 0:pack * E], xsl)

                for h in range(pack // G):
                    # mmA group: 2*G tiles of (L@X)^T into one PSUM bank
                    zp = zps.tile([P, 2 * G * P], f32)
                    for jj in range(G):
                        j = G * h + jj
                        for k in range(2):
                            m = 2 * jj + k
                            nc.tensor.matmul(
                                zp[:, m * P:(m + 1) * P],
                                xb[:, j * E + k * P: j * E + (k + 1) * P],
                                lt_sb[:],
                                start=True, stop=True,
                            )
                    zt = zsbp.tile([P, 2 * G * P], b_dt)
                    nc.vector.tensor_copy(zt[:], zp[:])
                    # mmB group: G projected j-tiles into one PSUM bank
                    op = ops.tile([P, G * E], f32)
                    for jj in range(G):
                        nc.tensor.matmul(
                            op[:, jj * E:(jj + 1) * E],
                            zt[:, 2 * jj * P:(2 * jj + 1) * P],
                            wt_sb[:, 0:E],
                            start=True, stop=False)
                        nc.tensor.matmul(
                            op[:, jj * E:(jj + 1) * E],
                            zt[:, (2 * jj + 1) * P:(2 * jj + 2) * P],
                            wt_sb[:, E:2 * E],
                            start=False, stop=True)
                    # eviction with fused per-partition bias add
                    ot = yout.tile([P, G * E], f32)
                    nc.scalar.activation(ot[:], op[:], ident, bias=beff_sb[:])
                    ysl = y[(tile0 + h * G) * P:(tile0 + (h + 1) * G) * P,
                            :].rearrange("(i p) e -> p i e", p=P)
                    # stores ride SWDGE so input prefetch never queues
                    # behind them on the HWDGE FIFO
                    nc.gpsimd.dma_start(ysl, ot[:])
                tile0 += pack
    nc.compile()
    return nc


def _np_dt(name):
    import ml_dtypes
    return {"bfloat16": ml_dtypes.bfloat16}.get(name, np.float32)


def _host_consts(W: np.ndarray, b: np.ndarray, mode: str):
    if mode in ("f32", "f32r"):
        a_np, b_np = np.float32, np.float32
    elif mode == "bf16":
        a_np, b_np = _np_dt("bfloat16"), _np_dt("bfloat16")
    else:
        raise ValueError(mode)

    idx = np.arange(P)
    blk = idx // BS
    LT = ((blk[:, None] == blk[None, :]) & (idx[:, None] < idx[None, :]))
    LT = np.ascontiguousarray(LT.astype(a_np))

    WT = W.T.astype(np.float32)                      # [E, T]
    WTP = np.concatenate([WT[0:P, :], WT[P:2 * P, :]], axis=1)  # [128, 512]
    WTP = np.ascontiguousarray(WTP.astype(b_np))

    be = b.astype(np.float64).copy()
    be[0] += be[0]
    BEFF = np.ascontiguousarray(
        be[idx % BS].astype(np.float32).reshape(P, 1))
    return LT, WTP, BEFF, a_np


def kernel(seq_vector, W, bias):
    mode = MODE
    x = np.asarray(seq_vector, dtype=np.float32)
    W = np.asarray(W, dtype=np.float32)
    b = np.asarray(bias, dtype=np.float32)
    assert x.shape == (N, S, E)

    LT, WTP, BEFF, a_np = _host_consts(W, b, mode)
    xs = np.ascontiguousarray(x.astype(a_np))

    if mode not in _cache:
        _cache[mode] = _build_nc(mode)
    nc = _cache[mode]

    in_maps = [
        {"x": xs[i], "lt": LT, "wt": WTP, "beff": BEFF}
        for i in range(NCORES)
    ]
    res = run_bass_kernel_spmd(nc, in_maps, core_ids=list(range(NCORES)))
    out = np.stack([r["y"] for r in res.results], axis=0)
    return out.reshape(N, S, E)
